# revision 2
# baseline (speedup 1.0000x reference)
"""GATv2 layer on 8 Trainium2 NeuronCores (Bass/Tile).

Strategy (target-major, fully static SPMD):
  * Host relabels nodes (degree-snake) so each 128-node window has ~equal
    incoming-edge mass, sorts edges by (relabeled) target, groups them into
    fixed 128-node windows, and splits each window's edges by source half
    (node id < NH) so gather indices fit int16 for dma_gather.
  * Node projections (left/right/values) are computed on device, sharded
    over cores (each core projects its 6272-node slice); the packed
    right||values table is AllGathered; `left` stays core-local in SBUF.
  * Per window: two dma_gather calls fetch right||values rows per edge.
    The combined pre-activation is built TRANSPOSED in PSUM
    (combT[f, e] = left_tab @ ohT + rv^T via identity-matmul), LeakyReLU'd
    on ScalarE in 4-chunk batches, and the attention dot-product is a PE
    matmul against a block-diagonal attn matrix (keeps DVE off the
    critical path). Edge bias (edge_features @ We + be) is precomputed on
    host and streamed as 8 cols/slot. Values use a (d, h)-permuted layout
    so the exp-broadcast multiply runs at DVE 2x mode.
  * Segment sums via one-hot matmuls (fp8 one-hot streams, half the HBM
    traffic of bf16); all edges of a target live in one window => no
    cross-core reduction needed.
  * Output = (num/den) @ Wo + bo, written transposed; host undoes layout.
"""
import numpy as np
import ml_dtypes

N_CORES = 8
N_NODES = 50000
NPAD = 50176          # 8 * 6272
PER_CORE = NPAD // N_CORES   # 6272
WIN = 128             # nodes per window
W_PER_CORE = PER_CORE // WIN  # 49
W_GLOBAL = NPAD // WIN        # 392
NH = NPAD // 2        # half-table rows (25088 < 32768 -> int16 ok)
IN_F = 256
OUT_F = 128
E_F = 64
H = 8
HD = 16
NEG_SLOPE = 0.2
bf16 = ml_dtypes.bfloat16
f8e4 = ml_dtypes.float8_e4m3

# permutation (h,d) -> (d,h) for the value/output feature order
PERM_DH = np.array([h * HD + d for d in range(HD) for h in range(H)], dtype=np.int64)


# ----------------------------------------------------------------------------
# host-side graph restructuring
# ----------------------------------------------------------------------------

def _host_prepare(node_features, edge_index, edge_features, Wl, bl, Wr, br,
                  We, be, attn_vector, Wv, bv, Wo, bo):
    s = np.asarray(edge_index[0], dtype=np.int64)
    t = np.asarray(edge_index[1], dtype=np.int64)
    E = s.shape[0]

    # --- degree-snake relabeling: balance incoming-edge mass per 128-window
    deg = np.bincount(t, minlength=NPAD).astype(np.int64)
    order = np.argsort(-deg, kind="stable")  # all NPAD ids (pads have deg 0)
    order = order[order < N_NODES] if N_NODES < NPAD else order
    # snake across W_GLOBAL windows
    new_id = np.empty(NPAD, dtype=np.int64)
    filln = np.zeros(W_GLOBAL, dtype=np.int64)
    w_seq = np.arange(len(order)) % (2 * W_GLOBAL)
    w_seq = np.where(w_seq < W_GLOBAL, w_seq, 2 * W_GLOBAL - 1 - w_seq)
    for node, w in zip(order, w_seq):
        new_id[node] = w * WIN + filln[w]
        filln[w] += 1
    # pad ids fill remaining slots
    spare = []
    for w in range(W_GLOBAL):
        for k in range(filln[w], WIN):
            spare.append(w * WIN + k)
    spare = np.array(spare, dtype=np.int64)
    pad_nodes = np.arange(N_NODES, NPAD)
    new_id[pad_nodes] = spare[:len(pad_nodes)] if len(pad_nodes) else spare[:0]
    inv_id = np.empty(NPAD, dtype=np.int64)
    inv_id[new_id] = np.arange(NPAD)

    ns = new_id[s]
    nt = new_id[t]

    # --- group edges by (window, source-half)
    w_of_edge = nt // WIN
    h_of_edge = (ns >= NH).astype(np.int64)
    key = w_of_edge * 2 + h_of_edge
    eorder = np.argsort(key, kind="stable")
    key_s = key[eorder]
    # counts per (window, half)
    cnt = np.bincount(key_s, minlength=2 * W_GLOBAL).reshape(W_GLOBAL, 2)
    D = int(np.ceil(cnt.max() / WIN))
    NI = D * WIN                 # slots per half
    SLOTS_W = 2 * NI             # slots per window
    EPC = W_PER_CORE * SLOTS_W   # padded edge slots per core
    C = 2 * D                    # chunks per window

    # slot assignment for each sorted edge
    starts = np.zeros(2 * W_GLOBAL + 1, dtype=np.int64)
    np.cumsum(cnt.reshape(-1), out=starts[1:])
    within = np.arange(E, dtype=np.int64) - starts[key_s]
    slot_global = key_s // 2 * SLOTS_W + (key_s % 2) * NI + within

    # staged per-slot arrays (global, then reshaped per core)
    TOT = W_GLOBAL * SLOTS_W
    tlw_slot = np.full(TOT, 200, dtype=np.int32)
    src_slot = np.zeros(TOT, dtype=np.int64)
    bias_slot = np.zeros((TOT, H), dtype=np.float32)
    es = eorder
    tlw_slot[slot_global] = (nt[es] % WIN).astype(np.int32)
    src_slot[slot_global] = ns[es] % NH
    # host-precomputed per-edge attention bias: ef @ We + be
    ebias = (np.asarray(edge_features, dtype=np.float32) @
             np.asarray(We, dtype=np.float32)) + np.asarray(be, dtype=np.float32)
    bias_slot[slot_global] = ebias[es]

    # one-hot streams (fp8), layout [W_GLOBAL, 128(part), C*128]
    tlw_wcp = tlw_slot.reshape(W_GLOBAL, C, WIN)  # [w, chunk, pos]
    n_ar = np.arange(WIN, dtype=np.int32)
    # onehotE[w, p, c, n] = (tlw[w, c, p] == n)   (partition = edge pos)
    ohE = (tlw_wcp.transpose(0, 2, 1)[:, :, :, None] == n_ar[None, None, None, :])
    ohE = ohE.astype(f8e4).reshape(W_GLOBAL, WIN, C * WIN)
    # onehotT[w, n, c, e] = (tlw[w, c, e] == n)   (partition = node)
    ohT = (n_ar[None, :, None, None] == tlw_wcp[:, None, :, :])
    ohT = ohT.astype(f8e4).reshape(W_GLOBAL, WIN, C * WIN)

    # per-window bias stream: [W, 128(pos-in-chunk), C*8]
    biasT = bias_slot.reshape(W_GLOBAL, C, WIN, H).transpose(0, 2, 1, 3)
    biasT = np.ascontiguousarray(biasT).reshape(W_GLOBAL, WIN, C * H).astype(bf16)

    # int16 gather indices, wrapped in 16 partitions replicated x8:
    # position i in a half -> idxs[[i%16, i//16]]
    src_wh = src_slot.reshape(W_GLOBAL, 2, NI)
    wrap = src_wh.reshape(W_GLOBAL, 2, NI // 16, 16).transpose(0, 1, 3, 2)
    wrap = wrap.reshape(W_GLOBAL, 2, 16, NI // 16).astype(np.int16)
    src16 = np.tile(wrap, (1, 1, 8, 1)).reshape(W_GLOBAL, 2, 128, NI // 16)
    src16 = np.ascontiguousarray(src16.transpose(0, 2, 1, 3)).reshape(
        W_GLOBAL, 128, 2 * (NI // 16))

    # node features (relabeled, transposed, +ones row, padded to 384 rows)
    nf = np.zeros((NPAD, IN_F), dtype=np.float32)
    nf[new_id[:N_NODES]] = np.asarray(node_features, dtype=np.float32)
    nfT = np.zeros((384, NPAD), dtype=np.float32)
    nfT[:IN_F] = nf.T
    nfT[IN_F] = 1.0
    nfT = nfT.astype(bf16)

    # weights
    def aug(Wm, bv_):
        a = np.zeros((384, Wm.shape[1]), dtype=np.float32)
        a[:IN_F] = np.asarray(Wm, dtype=np.float32)
        a[IN_F] = np.asarray(bv_, dtype=np.float32)
        return a
    # values permuted to (d, h) order so the exp broadcast is inner-contiguous
    Wvp = np.asarray(Wv, dtype=np.float32)[:, PERM_DH]
    bvp = np.asarray(bv, dtype=np.float32)[PERM_DH]
    Wrv = np.concatenate([aug(Wr, br), aug(Wvp, bvp)], axis=1).astype(bf16)  # [384, 256]
    Wla = aug(Wl, bl).astype(bf16)                                           # [384, 128]
    # block-diagonal attention matrix: Ablk[(h,d), h'] = attn[h,d] * (h==h')
    attn = np.asarray(attn_vector, dtype=np.float32)
    Ablk = np.zeros((128, H), dtype=np.float32)
    for h in range(H):
        Ablk[h * HD:(h + 1) * HD, h] = attn[h]
    Ablk = Ablk.astype(bf16)
    # Wo with rows permuted to (d, h) to match the aggregated layout
    Wo_p = np.asarray(Wo, dtype=np.float32)[PERM_DH, :].astype(bf16)
    bo_f = np.asarray(bo, dtype=np.float32).reshape(128, 1)

    host = dict(D=D, NI=NI, C=C, EPC=EPC, inv_id=inv_id, new_id=new_id)
    per_core = []
    for c in range(N_CORES):
        wlo, whi = c * W_PER_CORE, (c + 1) * W_PER_CORE
        per_core.append({
            "nfT": np.ascontiguousarray(nfT[:, c * PER_CORE:(c + 1) * PER_CORE]),
            "ohE": np.ascontiguousarray(ohE[wlo:whi]),
            "ohT": np.ascontiguousarray(ohT[wlo:whi]),
            "biasT": np.ascontiguousarray(biasT[wlo:whi]),
            "src16": np.ascontiguousarray(src16[wlo:whi]),
            "Wrv": Wrv, "Wla": Wla, "Ablk": Ablk,
            "Wo": Wo_p, "bo": bo_f,
        })
    return host, per_core


# ----------------------------------------------------------------------------
# device kernel
# ----------------------------------------------------------------------------

def _build_nc(D):
    import concourse.bass as bass
    import concourse.bacc as bacc
    import concourse.tile as tile
    from concourse import mybir
    from concourse.masks import make_identity

    f32 = mybir.dt.float32
    b16 = mybir.dt.bfloat16
    e4 = mybir.dt.float8e4
    i16 = mybir.dt.int16
    NI = D * WIN
    C = 2 * D
    CW = C * WIN
    LG = 4  # chunks per LeakyReLU batch (one PSUM bank = 512 f32 cols)

    nc = bacc.Bacc("TRN2", num_devices=N_CORES, debug=False)
    d_nfT = nc.dram_tensor("nfT", [384, PER_CORE], b16, kind="ExternalInput").ap()
    d_ohE = nc.dram_tensor("ohE", [W_PER_CORE, 128, CW], e4, kind="ExternalInput").ap()
    d_ohT = nc.dram_tensor("ohT", [W_PER_CORE, 128, CW], e4, kind="ExternalInput").ap()
    d_biasT = nc.dram_tensor("biasT", [W_PER_CORE, 128, C * H], b16, kind="ExternalInput").ap()
    d_src = nc.dram_tensor("src16", [W_PER_CORE, 128, 2 * (NI // 16)], i16, kind="ExternalInput").ap()
    d_Wrv = nc.dram_tensor("Wrv", [384, 256], b16, kind="ExternalInput").ap()
    d_Wla = nc.dram_tensor("Wla", [384, 128], b16, kind="ExternalInput").ap()
    d_Ablk = nc.dram_tensor("Ablk", [128, H], b16, kind="ExternalInput").ap()
    d_Wo = nc.dram_tensor("Wo", [128, 128], b16, kind="ExternalInput").ap()
    d_bo = nc.dram_tensor("bo", [128, 1], f32, kind="ExternalInput").ap()
    d_out = nc.dram_tensor("outT", [128, PER_CORE], f32, kind="ExternalOutput").ap()

    with tile.TileContext(nc) as tc:
        with (
            tc.tile_pool(name="const", bufs=1) as cons,
            tc.tile_pool(name="tbl", bufs=3) as tblp,
            tc.tile_pool(name="win", bufs=3) as winp,
            tc.tile_pool(name="psum", bufs=2, space="PSUM") as psp,
            tc.tile_pool(name="dram", bufs=1, space="DRAM") as dram,
        ):
            # ---- constants
            Wrv_sb = cons.tile([128, 3, 256], b16)
            nc.sync.dma_start(out=Wrv_sb[:], in_=d_Wrv.rearrange("(j p) n -> p j n", p=128))
            Wla_sb = cons.tile([128, 3, 128], b16)
            nc.sync.dma_start(out=Wla_sb[:], in_=d_Wla.rearrange("(j p) n -> p j n", p=128))
            Ablk_sb = cons.tile([128, H], b16)
            nc.sync.dma_start(out=Ablk_sb[:], in_=d_Ablk[:, :])
            Wo_sb = cons.tile([128, 128], b16)
            nc.sync.dma_start(out=Wo_sb[:], in_=d_Wo[:, :])
            bo_sb = cons.tile([128, 1], f32)
            nc.sync.dma_start(out=bo_sb[:], in_=d_bo[:, :])
            ident = cons.tile([128, 128], b16)
            make_identity(nc, ident[:])
            left_tab = cons.tile([128, W_PER_CORE * 128], b16)

            # ---- table phase: project this core's node slice
            # (KREPS>1 replicates the whole kernel body for slope-based timing)
            import os
            _kreps = int(os.environ.get("KREPS", "1"))
            rv_loc = dram.tile([PER_CORE, 256], b16)
            rv_full = dram.tile([NPAD, 256], b16)
          # replication loop (timing only; KREPS=1 in production)
          # fmt: off
            for _rep in range(_kreps):
              for tti in range(W_PER_CORE):
                nf3 = tblp.tile([128, 3, 128], b16, tag="nf3")
                nc.sync.dma_start(
                    out=nf3[:],
                    in_=d_nfT.rearrange("(j p) n -> p j n", p=128)[:, :, tti * 128:(tti + 1) * 128])
                ps_rv = psp.tile([128, 256], f32, tag="comb")
                ps_l = psp.tile([128, 128], f32, tag="score")
                for j in range(3):
                    nc.tensor.matmul(out=ps_rv[:], lhsT=nf3[:, j, :], rhs=Wrv_sb[:, j, :],
                                     start=(j == 0), stop=(j == 2))
                for j in range(3):
                    nc.tensor.matmul(out=ps_l[:], lhsT=nf3[:, j, :], rhs=Wla_sb[:, j, :],
                                     start=(j == 0), stop=(j == 2))
                rv_sb = tblp.tile([128, 256], b16, tag="rvsb")
                nc.vector.tensor_copy(out=rv_sb[:], in_=ps_rv[:])
                nc.vector.tensor_copy(out=left_tab[:, tti * 128:(tti + 1) * 128], in_=ps_l[:])
                nc.sync.dma_start(out=rv_loc[tti * 128:(tti + 1) * 128, :], in_=rv_sb[:])

            if os.environ.get("SIMMODE", "0") == "1":
                nc.sync.dma_start(out=rv_full[:PER_CORE, :], in_=rv_loc[:, :])
            else:
                nc.gpsimd.collective_compute(
                    "AllGather", mybir.AluOpType.bypass,
                    replica_groups=[list(range(N_CORES))],
                    ins=[rv_loc[:].opt()], outs=[rv_full[:].opt()],
                )

            # ---- edge phase
            for _rep in range(_kreps):
              for w in range(W_PER_CORE):
                ohT_sb = winp.tile([128, CW], e4, tag="ohT")
                nc.sync.dma_start(out=ohT_sb[:], in_=d_ohT[w, :, :])
                ohE_sb = winp.tile([128, CW], e4, tag="ohE")
                nc.sync.dma_start(out=ohE_sb[:], in_=d_ohE[w, :, :])
                bias_sb = winp.tile([128, C * H], b16, tag="bias")
                nc.sync.dma_start(out=bias_sb[:], in_=d_biasT[w, :, :])
                src_sb = winp.tile([128, 2 * (NI // 16)], i16, tag="src")
                nc.sync.dma_start(out=src_sb[:], in_=d_src[w, :, :])

                rv_g = winp.tile([128, C, 256], b16, tag="rvg")
                nc.gpsimd.dma_gather(
                    out_ap=rv_g[:, :D, :], in_ap=rv_full[:NH, :],
                    idxs_ap=src_sb[:, :NI // 16],
                    num_idxs=NI, num_idxs_reg=NI, elem_size=256, single_packet=False)
                nc.gpsimd.dma_gather(
                    out_ap=rv_g[:, D:, :], in_ap=rv_full[NH:, :],
                    idxs_ap=src_sb[:, NI // 16:],
                    num_idxs=NI, num_idxs_reg=NI, elem_size=256, single_packet=False)

                # scores for the whole window accumulate here: [e, (c, h)]
                ps_score = psp.tile([128, C * H], f32, tag="score")
                left_w = left_tab[:, w * 128:(w + 1) * 128]

                for g0 in range(0, C, LG):
                    gs = min(LG, C - g0)
                    ps_comb = psp.tile([128, LG * 128], f32, tag="comb")
                    for k in range(gs):
                        cc = g0 + k
                        # combT[f, e] = left[target] + right[source], transposed
                        nc.tensor.matmul(out=ps_comb[:, k * 128:(k + 1) * 128],
                                         lhsT=left_w, rhs=ohT_sb[:, cc * 128:(cc + 1) * 128],
                                         start=True, stop=False)
                        nc.tensor.matmul(out=ps_comb[:, k * 128:(k + 1) * 128],
                                         lhsT=rv_g[:, cc, 0:128], rhs=ident[:],
                                         start=False, stop=True)
                    act_g = winp.tile([128, LG * 128], b16, tag="act")
                    nc.scalar.activation(out=act_g[:, :gs * 128], in_=ps_comb[:, :gs * 128],
                                         func=mybir.ActivationFunctionType.Lrelu,
                                         alpha=NEG_SLOPE)
                    for k in range(gs):
                        cc = g0 + k
                        # score[e, h] = sum_(h,d) actT[(h,d), e] * Ablk[(h,d), h]
                        nc.tensor.matmul(out=ps_score[:, cc * H:(cc + 1) * H],
                                         lhsT=act_g[:, k * 128:(k + 1) * 128],
                                         rhs=Ablk_sb[:], start=True, stop=True)

                # add the per-edge bias, exponentiate
                scores_sb = winp.tile([128, C * H], f32, tag="scores")
                nc.vector.tensor_tensor(out=scores_sb[:], in0=ps_score[:], in1=bias_sb[:],
                                        op=mybir.AluOpType.add)
                exp_sb = winp.tile([128, C, H], b16, tag="exp")
                nc.scalar.activation(out=exp_sb[:], in_=scores_sb[:].rearrange("p (c h) -> p c h", h=H),
                                     func=mybir.ActivationFunctionType.Exp)

                # weighted values (values are (d,h)-ordered -> 2x DVE mode)
                wgt = winp.tile([128, C, 136], b16, tag="wgt")
                nc.vector.tensor_copy(out=wgt[:, :, 128:136], in_=exp_sb[:])
                for hf in range(2):  # per gather half (D chunks each)
                    eh = exp_sb[:, hf * D:(hf + 1) * D, :]
                    nc.vector.tensor_tensor(
                        out=wgt[:, hf * D:(hf + 1) * D, 0:128].rearrange(
                            "p c (d h) -> p c d h", d=HD),
                        in0=rv_g[:, hf * D:(hf + 1) * D, 128:256].rearrange(
                            "p c (d h) -> p c d h", d=HD),
                        in1=bass.AP(tensor=eh.tensor, offset=eh.offset,
                                    ap=[[eh.ap[0][0], 128], [eh.ap[1][0], D],
                                        [0, HD], [1, H]]),
                        op=mybir.AluOpType.mult)

                ps_agg = psp.tile([128, 136], f32, tag="agg")
                for cc in range(C):
                    nc.tensor.matmul(out=ps_agg[:], lhsT=ohE_sb[:, cc * 128:(cc + 1) * 128],
                                     rhs=wgt[:, cc, :], start=(cc == 0), stop=(cc == C - 1))

                # ---- finalize window: out = (num/den) @ Wo + bo (transposed)
                den = winp.tile([128, H], f32, tag="den")
                nc.vector.tensor_scalar_add(out=den[:], in0=ps_agg[:, 128:136], scalar1=1e-10)
                rec = winp.tile([128, H], f32, tag="rec")
                nc.vector.reciprocal(out=rec[:], in_=den[:])
                h_sb = winp.tile([128, 128], b16, tag="hsb")
                rec_ap = rec[:]
                nc.vector.tensor_tensor(
                    out=h_sb[:].rearrange("p (d h) -> p d h", d=HD),
                    in0=ps_agg[:, 0:128].rearrange("p (d h) -> p d h", d=HD),
                    in1=bass.AP(tensor=rec_ap.tensor, offset=rec_ap.offset,
                                ap=[[rec_ap.ap[0][0], 128], [0, HD], [1, H]]),
                    op=mybir.AluOpType.mult)
                ps_T = psp.tile([128, 128], b16, tag="fin")
                nc.tensor.transpose(out=ps_T[:], in_=h_sb[:], identity=ident[:])
                hT_sb = winp.tile([128, 128], b16, tag="hTsb")
                nc.vector.tensor_copy(out=hT_sb[:], in_=ps_T[:])
                ps_out = psp.tile([128, 128], f32, tag="fin")
                nc.tensor.matmul(out=ps_out[:], lhsT=Wo_sb[:], rhs=hT_sb[:],
                                 start=True, stop=True)
                out_sb = winp.tile([128, 128], f32, tag="osb")
                nc.scalar.activation(out=out_sb[:], in_=ps_out[:],
                                     func=mybir.ActivationFunctionType.Identity,
                                     bias=bo_sb[:])
                nc.sync.dma_start(out=d_out[:, w * 128:(w + 1) * 128], in_=out_sb[:])
    nc.compile()
    return nc


# ----------------------------------------------------------------------------
# inline SPMD runner (self-contained; mirrors concourse.bass2jax.run_bass_via_pjrt)
# ----------------------------------------------------------------------------

def _run_spmd(nc, in_maps):
    import jax
    import numpy as _np
    from jax.sharding import Mesh, PartitionSpec
    from jax.experimental.shard_map import shard_map
    import concourse.mybir as mybir
    from concourse.bass2jax import install_neuronx_cc_hook, _bass_exec_p, partition_id_tensor

    install_neuronx_cc_hook()
    partition_name = nc.partition_id_tensor.name if nc.partition_id_tensor else None
    in_names, out_names, out_avals, zero_outs = [], [], [], []
    for alloc in nc.m.functions[0].allocations:
        if not isinstance(alloc, mybir.MemoryLocationSet):
            continue
        name = alloc.memorylocations[0].name
        if alloc.kind == "ExternalInput":
            if name != partition_name:
                in_names.append(name)
        elif alloc.kind == "ExternalOutput":
            out_names.append(name)
            shape = tuple(alloc.tensor_shape)
            dtype = mybir.dt.np(alloc.dtype)
            out_avals.append(jax.core.ShapedArray(shape, dtype))
            zero_outs.append(_np.zeros(shape, dtype))
    n_params = len(in_names)
    all_in_names = list(in_names) + list(out_names)
    if partition_name is not None:
        all_in_names.append(partition_name)

    def _body(*args):
        operands = list(args)
        if partition_name is not None:
            operands.append(partition_id_tensor())
        outs = _bass_exec_p.bind(
            *operands,
            out_avals=tuple(out_avals),
            in_names=tuple(all_in_names),
            out_names=tuple(out_names),
            lowering_input_output_aliases=(),
            sim_require_finite=False,
            sim_require_nnan=False,
            nc=nc,
        )
        return tuple(outs)

    donate = tuple(range(n_params, n_params + len(out_avals)))
    devices = jax.devices()[:N_CORES]
    mesh = Mesh(_np.asarray(devices), ("core",))
    in_specs = (PartitionSpec("core"),) * (n_params + len(out_avals))
    out_specs = (PartitionSpec("core"),) * len(out_names)
    fn = jax.jit(shard_map(_body, mesh=mesh, in_specs=in_specs,
                           out_specs=out_specs, check_rep=False),
                 donate_argnums=donate, keep_unused=True)
    ins = []
    for nm in in_names:
        cat = _np.concatenate([_np.asarray(m[nm]) for m in in_maps], axis=0)
        ins.append(jax.device_put(cat, jax.sharding.NamedSharding(mesh, PartitionSpec("core"))))
    zouts = []
    for z in zero_outs:
        cat = _np.concatenate([z] * N_CORES, axis=0)
        zouts.append(jax.device_put(cat, jax.sharding.NamedSharding(mesh, PartitionSpec("core"))))
    outs = fn(*ins, *zouts)
    outs = [_np.asarray(o) for o in outs]
    per_core = []
    for c in range(N_CORES):
        d = {}
        for i, nm in enumerate(out_names):
            full = outs[i]
            rows = full.shape[0] // N_CORES
            d[nm] = full[c * rows:(c + 1) * rows]
        per_core.append(d)
    return per_core


_CACHE = {}


def kernel(node_features, edge_index, edge_features,
           Wl, bl, Wr, br, We, be, attn_vector, Wv, bv, Wo, bo):
    host, per_core = _host_prepare(node_features, edge_index, edge_features,
                                   Wl, bl, Wr, br, We, be, attn_vector,
                                   Wv, bv, Wo, bo)
    D = host["D"]
    if D not in _CACHE:
        _CACHE[D] = _build_nc(D)
    nc = _CACHE[D]

    res = _run_spmd(nc, per_core)
    outT = np.concatenate([res[c]["outT"] for c in range(N_CORES)], axis=1)  # [128, NPAD]
    out_relab = outT.T  # [NPAD, 128]
    out = out_relab[host["new_id"][:N_NODES]]
    return np.ascontiguousarray(out, dtype=np.float32)


# revision 10
# speedup vs baseline: 1.1363x; 1.1363x over previous
"""GATv2 layer on 8 Trainium2 NeuronCores (Bass/Tile).

Strategy (target-major, fully static SPMD):
  * Host relabels nodes (degree-snake) so each 128-node window has ~equal
    incoming-edge mass, sorts edges by (relabeled) target, groups them into
    fixed 128-node windows, and splits each window's edges by source half
    (node id < NH) so gather indices fit int16 for dma_gather.
  * Node projections (left/right/values) are computed on device, sharded
    over cores (each core projects its 6272-node slice); the packed
    right||values table is AllGathered; `left` stays core-local in SBUF.
  * Per window: one packed byte-blob DMA brings the fp8 one-hots, bf16
    edge-bias (host-precomputed ef@We+be) and int16 gather indices; two
    dma_gather calls fetch right||values rows per edge.  The combined
    pre-activation is built TRANSPOSED in PSUM (combT[f,e] = left_tab @ ohT
    + rv^T via identity-matmul), Prelu'd on ScalarE in 4-chunk batches
    (Prelu shares the `exp_and_others` table set with Exp -> no act-table
    thrash), and the attention dot-product is a PE matmul against a
    block-diagonal attn matrix.  Values use a (d, h)-permuted layout so the
    exp-broadcast multiply runs at DVE 2x mode.
  * Segment sums via one-hot matmuls (fp8 one-hot streams); all edges of a
    target live in one window => no cross-core reduction needed.
  * Output = (num/den) @ Wo + bo, written transposed; host undoes layout.
"""
import numpy as np
import ml_dtypes

N_CORES = 8
N_NODES = 50000
NPAD = 50176          # 8 * 6272
PER_CORE = NPAD // N_CORES   # 6272
WIN = 128             # nodes per window
W_PER_CORE = PER_CORE // WIN  # 49
W_GLOBAL = NPAD // WIN        # 392
NH = NPAD // 2        # half-table rows (25088 < 32768 -> int16 ok)
IN_F = 256
OUT_F = 128
E_F = 64
H = 8
HD = 16
NEG_SLOPE = 0.2
bf16 = ml_dtypes.bfloat16
f8e4 = ml_dtypes.float8_e4m3

# permutation (h,d) -> (d,h) for the value/output feature order
PERM_DH = np.array([h * HD + d for d in range(HD) for h in range(H)], dtype=np.int64)


# ----------------------------------------------------------------------------
# host-side graph restructuring
# ----------------------------------------------------------------------------

def _host_prepare(node_features, edge_index, edge_features, Wl, bl, Wr, br,
                  We, be, attn_vector, Wv, bv, Wo, bo):
    s = np.asarray(edge_index[0], dtype=np.int64)
    t = np.asarray(edge_index[1], dtype=np.int64)
    E = s.shape[0]

    # --- degree-snake relabeling: balance incoming-edge mass per 128-window
    deg = np.bincount(t, minlength=NPAD).astype(np.int64)
    order = np.argsort(-deg, kind="stable")  # all NPAD ids (pads have deg 0)
    order = order[order < N_NODES] if N_NODES < NPAD else order
    # snake across W_GLOBAL windows
    new_id = np.empty(NPAD, dtype=np.int64)
    filln = np.zeros(W_GLOBAL, dtype=np.int64)
    w_seq = np.arange(len(order)) % (2 * W_GLOBAL)
    w_seq = np.where(w_seq < W_GLOBAL, w_seq, 2 * W_GLOBAL - 1 - w_seq)
    for node, w in zip(order, w_seq):
        new_id[node] = w * WIN + filln[w]
        filln[w] += 1
    # pad ids fill remaining slots
    spare = []
    for w in range(W_GLOBAL):
        for k in range(filln[w], WIN):
            spare.append(w * WIN + k)
    spare = np.array(spare, dtype=np.int64)
    pad_nodes = np.arange(N_NODES, NPAD)
    new_id[pad_nodes] = spare[:len(pad_nodes)] if len(pad_nodes) else spare[:0]
    inv_id = np.empty(NPAD, dtype=np.int64)
    inv_id[new_id] = np.arange(NPAD)

    ns = new_id[s]
    nt = new_id[t]

    # --- group edges by (window, source-half)
    w_of_edge = nt // WIN
    h_of_edge = (ns >= NH).astype(np.int64)
    key = w_of_edge * 2 + h_of_edge
    eorder = np.argsort(key, kind="stable")
    key_s = key[eorder]
    # counts per (window, half)
    cnt = np.bincount(key_s, minlength=2 * W_GLOBAL).reshape(W_GLOBAL, 2)
    D = int(np.ceil(cnt.max() / WIN))
    NI = D * WIN                 # slots per half
    SLOTS_W = 2 * NI             # slots per window
    EPC = W_PER_CORE * SLOTS_W   # padded edge slots per core
    C = 2 * D                    # chunks per window

    # slot assignment for each sorted edge
    starts = np.zeros(2 * W_GLOBAL + 1, dtype=np.int64)
    np.cumsum(cnt.reshape(-1), out=starts[1:])
    within = np.arange(E, dtype=np.int64) - starts[key_s]
    slot_global = key_s // 2 * SLOTS_W + (key_s % 2) * NI + within

    # staged per-slot arrays (global, then reshaped per core)
    TOT = W_GLOBAL * SLOTS_W
    tlw_slot = np.full(TOT, 200, dtype=np.int32)
    src_slot = np.zeros(TOT, dtype=np.int64)
    bias_slot = np.zeros((TOT, H), dtype=np.float32)
    es = eorder
    tlw_slot[slot_global] = (nt[es] % WIN).astype(np.int32)
    src_slot[slot_global] = ns[es] % NH
    # host-precomputed per-edge attention bias: ef @ We + be
    ebias = (np.asarray(edge_features, dtype=np.float32) @
             np.asarray(We, dtype=np.float32)) + np.asarray(be, dtype=np.float32)
    bias_slot[slot_global] = ebias[es]

    # one-hot streams (fp8), layout [W_GLOBAL, 128(part), C*128]
    tlw_wcp = tlw_slot.reshape(W_GLOBAL, C, WIN)  # [w, chunk, pos]
    n_ar = np.arange(WIN, dtype=np.int32)
    # onehotE[w, p, c, n] = (tlw[w, c, p] == n)   (partition = edge pos)
    ohE = (tlw_wcp.transpose(0, 2, 1)[:, :, :, None] == n_ar[None, None, None, :])
    ohE = ohE.astype(f8e4).reshape(W_GLOBAL, WIN, C * WIN)
    # onehotT[w, n, c, e] = (tlw[w, c, e] == n)   (partition = node)
    ohT = (n_ar[None, :, None, None] == tlw_wcp[:, None, :, :])
    ohT = ohT.astype(f8e4).reshape(W_GLOBAL, WIN, C * WIN)

    # per-window bias stream: [W, 128(pos-in-chunk), C*8] bf16
    biasT = bias_slot.reshape(W_GLOBAL, C, WIN, H).transpose(0, 2, 1, 3)
    biasT = np.ascontiguousarray(biasT).reshape(W_GLOBAL, WIN, C * H).astype(bf16)

    # int16 gather indices, wrapped in 16 partitions replicated x8:
    # position i in a half -> idxs[[i%16, i//16]]
    src_wh = src_slot.reshape(W_GLOBAL, 2, NI)
    wrap = src_wh.reshape(W_GLOBAL, 2, NI // 16, 16).transpose(0, 1, 3, 2)
    wrap = wrap.reshape(W_GLOBAL, 2, 16, NI // 16).astype(np.int16)
    src16 = np.tile(wrap, (1, 1, 8, 1)).reshape(W_GLOBAL, 2, 128, NI // 16)
    src16 = np.ascontiguousarray(src16.transpose(0, 2, 1, 3)).reshape(
        W_GLOBAL, 128, 2 * (NI // 16))

    # pack (ohT | ohE | biasT) into one byte blob per window; gather indices
    # ship separately (resident in SBUF for the whole kernel -> gathers can
    # prefetch without waiting for the big per-window stream)
    blob = np.concatenate([
        ohT.view(np.uint8),
        ohE.view(np.uint8),
        biasT.view(np.uint8).reshape(W_GLOBAL, WIN, C * H * 2),
    ], axis=2)  # [W, 128, NB]

    # node features (relabeled, transposed, +ones row, padded to 384 rows)
    nf = np.zeros((NPAD, IN_F), dtype=np.float32)
    nf[new_id[:N_NODES]] = np.asarray(node_features, dtype=np.float32)
    nfT = np.zeros((384, NPAD), dtype=np.float32)
    nfT[:IN_F] = nf.T
    nfT[IN_F] = 1.0
    nfT = nfT.astype(bf16)

    # weights
    def aug(Wm, bv_):
        a = np.zeros((384, Wm.shape[1]), dtype=np.float32)
        a[:IN_F] = np.asarray(Wm, dtype=np.float32)
        a[IN_F] = np.asarray(bv_, dtype=np.float32)
        return a
    # values permuted to (d, h) order so the exp broadcast is inner-contiguous
    Wvp = np.asarray(Wv, dtype=np.float32)[:, PERM_DH]
    bvp = np.asarray(bv, dtype=np.float32)[PERM_DH]
    # packed projection: [right | values' | left] -> [384, 384]
    Wall = np.concatenate([aug(Wr, br), aug(Wvp, bvp), aug(Wl, bl)],
                          axis=1).astype(bf16)
    # block-diagonal attention matrix: Ablk[(h,d), h'] = attn[h,d] * (h==h')
    attn = np.asarray(attn_vector, dtype=np.float32)
    Ablk = np.zeros((128, H), dtype=np.float32)
    for h in range(H):
        Ablk[h * HD:(h + 1) * HD, h] = attn[h]
    Ablk = Ablk.astype(bf16)
    # Wo with rows permuted to (d, h) to match the aggregated layout
    Wo_p = np.asarray(Wo, dtype=np.float32)[PERM_DH, :].astype(bf16)
    bo_f = np.asarray(bo, dtype=np.float32).reshape(128, 1)

    host = dict(D=D, NI=NI, C=C, EPC=EPC, inv_id=inv_id, new_id=new_id)
    per_core = []
    for c in range(N_CORES):
        wlo, whi = c * W_PER_CORE, (c + 1) * W_PER_CORE
        src_c = src16[wlo:whi].transpose(1, 0, 2).reshape(
            128, W_PER_CORE * 2 * (NI // 16))
        per_core.append({
            "nfT": np.ascontiguousarray(nfT[:, c * PER_CORE:(c + 1) * PER_CORE]),
            "blob": np.ascontiguousarray(blob[wlo:whi]),
            "srcA": np.ascontiguousarray(src_c),
            "Wall": Wall, "Ablk": Ablk,
            "Wo": Wo_p, "bo": bo_f,
        })
    return host, per_core


# ----------------------------------------------------------------------------
# device kernel
# ----------------------------------------------------------------------------

def _build_nc(D):
    import concourse.bass as bass
    import concourse.bacc as bacc
    import concourse.tile as tile
    from concourse import mybir
    from concourse.masks import make_identity

    f32 = mybir.dt.float32
    b16 = mybir.dt.bfloat16
    e4 = mybir.dt.float8e4
    i16 = mybir.dt.int16
    i8 = mybir.dt.int8
    NI = D * WIN
    C = 2 * D
    CW = C * WIN
    LG = 6  # chunks per Prelu batch (2 PSUM banks)
    # byte offsets inside the per-window blob
    OFF_OHT = 0
    OFF_OHE = CW
    OFF_BIAS = 2 * CW
    NB = OFF_BIAS + 2 * C * H
    SRCW = 2 * (NI // 16)  # int16 idx cols per window

    import os
    GKIND = os.environ.get("GKIND", "gather")

    nc = bacc.Bacc("TRN2", num_devices=N_CORES, debug=False)
    d_nfT = nc.dram_tensor("nfT", [384, PER_CORE], b16, kind="ExternalInput").ap()
    d_blob = nc.dram_tensor("blob", [W_PER_CORE, 128, NB], i8, kind="ExternalInput").ap()
    d_src = nc.dram_tensor("srcA", [128, W_PER_CORE * SRCW], i16, kind="ExternalInput").ap()
    d_Wall = nc.dram_tensor("Wall", [384, 384], b16, kind="ExternalInput").ap()
    d_Ablk = nc.dram_tensor("Ablk", [128, H], b16, kind="ExternalInput").ap()
    d_Wo = nc.dram_tensor("Wo", [128, 128], b16, kind="ExternalInput").ap()
    d_bo = nc.dram_tensor("bo", [128, 1], f32, kind="ExternalInput").ap()
    d_out = nc.dram_tensor("outT", [128, PER_CORE], f32, kind="ExternalOutput").ap()

    with tile.TileContext(nc) as tc:
        with (
            tc.tile_pool(name="const", bufs=1) as cons,
            tc.tile_pool(name="tbl", bufs=3) as tblp,
            tc.tile_pool(name="win", bufs=4) as winp,
            tc.tile_pool(name="psum", bufs=2, space="PSUM") as psp,
            tc.tile_pool(name="dram", bufs=1, space="DRAM") as dram,
        ):
            # ---- constants
            Wall_sb = cons.tile([128, 3, 384], b16)
            nc.sync.dma_start(out=Wall_sb[:], in_=d_Wall.rearrange("(j p) n -> p j n", p=128))
            Ablk_sb = cons.tile([128, H], b16)
            nc.sync.dma_start(out=Ablk_sb[:], in_=d_Ablk[:, :])
            Wo_sb = cons.tile([128, 128], b16)
            nc.sync.dma_start(out=Wo_sb[:], in_=d_Wo[:, :])
            bo_sb = cons.tile([128, 1], f32)
            nc.sync.dma_start(out=bo_sb[:], in_=d_bo[:, :])
            ident = cons.tile([128, 128], b16)
            make_identity(nc, ident[:])
            left_tab = cons.tile([128, W_PER_CORE * 128], b16)
            src_all = cons.tile([128, W_PER_CORE * SRCW], i16)
            nc.sync.dma_start(out=src_all[:], in_=d_src[:, :])

            # ---- table phase: project this core's node slice
            # (KREPS>1 replicates the whole kernel body for slope-based timing)
            _kreps = int(os.environ.get("KREPS", "1"))
            rv_loc = dram.tile([PER_CORE, 256], b16)
            rv_full = dram.tile([NPAD, 256], b16)
          # replication loop (timing only; KREPS=1 in production)
          # fmt: off
            for _rep in range(_kreps):
              for t0 in range(0, W_PER_CORE, 2):
                tn = min(2, W_PER_CORE - t0)
                nfx = tblp.tile([128, 3, 2 * 128], b16, tag="nfx")
                nc.sync.dma_start(
                    out=nfx[:, :, :tn * 128],
                    in_=d_nfT.rearrange("(j p) n -> p j n", p=128)[:, :, t0 * 128:(t0 + tn) * 128])
                rv2 = tblp.tile([128, 2, 256], b16, tag="rv2")
                for u in range(tn):
                    tti = t0 + u
                    ps_all = psp.tile([128, 384], f32, tag="comb")
                    for j in range(3):
                        nc.tensor.matmul(out=ps_all[:], lhsT=nfx[:, j, u * 128:(u + 1) * 128],
                                         rhs=Wall_sb[:, j, :], start=(j == 0), stop=(j == 2))
                    nc.vector.tensor_copy(out=rv2[:, u, :], in_=ps_all[:, 0:256])
                    nc.vector.tensor_copy(out=left_tab[:, tti * 128:(tti + 1) * 128],
                                          in_=ps_all[:, 256:384])
                nc.sync.dma_start(
                    out=rv_loc[t0 * 128:(t0 + tn) * 128, :].rearrange("(u p) n -> p u n", p=128),
                    in_=rv2[:, :tn, :])

            if os.environ.get("SIMMODE", "0") == "1":
                nc.sync.dma_start(out=rv_full[:PER_CORE, :], in_=rv_loc[:, :])
            else:
                nc.gpsimd.collective_compute(
                    "AllGather", mybir.AluOpType.bypass,
                    replica_groups=[list(range(N_CORES))],
                    ins=[rv_loc[:].opt()], outs=[rv_full[:].opt()],
                )

            # ---- edge phase
            for _rep in range(_kreps):
              for w in range(W_PER_CORE):
                blob_sb = winp.tile([128, NB], i8, tag="blob")
                nc.sync.dma_start(out=blob_sb[:], in_=d_blob[w, :, :])
                ohT_v = blob_sb[:, OFF_OHT:OFF_OHT + CW].bitcast(e4)
                ohE_v = blob_sb[:, OFF_OHE:OFF_OHE + CW].bitcast(e4)
                bias_v = blob_sb[:, OFF_BIAS:OFF_BIAS + 2 * C * H].bitcast(b16)
                src_v = src_all[:, w * SRCW:(w + 1) * SRCW]

                rv_g = winp.tile([128, C, 256], b16, tag="rvg")
                if GKIND == "seq":
                    # ablation: same bytes, sequential instead of gathered
                    b0 = (w * 2 * NI) % (NPAD - 2 * NI)
                    nc.sync.dma_start(
                        out=rv_g[:],
                        in_=rv_full[b0:b0 + 2 * NI, :].rearrange(
                            "(c p) n -> p c n", p=128))
                else:
                    nc.gpsimd.dma_gather(
                        out_ap=rv_g[:, :D, :], in_ap=rv_full[:NH, :],
                        idxs_ap=src_v[:, :NI // 16],
                        num_idxs=NI, num_idxs_reg=NI, elem_size=256, single_packet=False)
                    nc.gpsimd.dma_gather(
                        out_ap=rv_g[:, D:, :], in_ap=rv_full[NH:, :],
                        idxs_ap=src_v[:, NI // 16:],
                        num_idxs=NI, num_idxs_reg=NI, elem_size=256, single_packet=False)

                # scores for the whole window accumulate here: [e, (c, h)]
                ps_score = psp.tile([128, C * H], f32, tag="sa")
                left_w = left_tab[:, w * 128:(w + 1) * 128]

                for g0 in range(0, C, LG):
                    gs = min(LG, C - g0)
                    ps_comb = psp.tile([128, LG * 128], f32, tag="comb")
                    # combT[f, e] = left[target] (batched matmuls over gs chunks)
                    for m0 in range(0, gs * 128, 512):
                        m1 = min(gs * 128, m0 + 512)
                        nc.tensor.matmul(out=ps_comb[:, m0:m1], lhsT=left_w,
                                         rhs=ohT_v[:, g0 * 128 + m0:g0 * 128 + m1],
                                         start=True, stop=False, skip_group_check=True)
                    for k in range(gs):
                        cc = g0 + k
                        # += right[source]^T via identity matmul
                        nc.tensor.matmul(out=ps_comb[:, k * 128:(k + 1) * 128],
                                         lhsT=rv_g[:, cc, 0:128], rhs=ident[:],
                                         start=False, stop=True, skip_group_check=True)
                    act_g = winp.tile([128, LG * 128], b16, tag="act")
                    nc.scalar.activation(out=act_g[:, :gs * 128], in_=ps_comb[:, :gs * 128],
                                         func=mybir.ActivationFunctionType.Prelu,
                                         alpha=NEG_SLOPE)
                    for k in range(gs):
                        cc = g0 + k
                        # score[e, h] = sum_(h,d) actT[(h,d), e] * Ablk[(h,d), h]
                        nc.tensor.matmul(out=ps_score[:, cc * H:(cc + 1) * H],
                                         lhsT=act_g[:, k * 128:(k + 1) * 128],
                                         rhs=Ablk_sb[:], start=True, stop=True)

                # add the per-edge bias, exponentiate
                scores_sb = winp.tile([128, C * H], f32, tag="scores")
                nc.vector.tensor_tensor(out=scores_sb[:], in0=ps_score[:], in1=bias_v,
                                        op=mybir.AluOpType.add)
                exp_sb = winp.tile([128, C, H], b16, tag="exp")
                nc.scalar.activation(out=exp_sb[:], in_=scores_sb[:].rearrange("p (c h) -> p c h", h=H),
                                     func=mybir.ActivationFunctionType.Exp)

                # weighted values (values are (d,h)-ordered -> 2x DVE mode)
                wgt = winp.tile([128, C, 136], b16, tag="wgt")
                nc.vector.tensor_copy(out=wgt[:, :, 128:136], in_=exp_sb[:])
                for hf in range(2):  # per gather half (D chunks each)
                    eh = exp_sb[:, hf * D:(hf + 1) * D, :]
                    nc.vector.tensor_tensor(
                        out=wgt[:, hf * D:(hf + 1) * D, 0:128].rearrange(
                            "p c (d h) -> p c d h", d=HD),
                        in0=rv_g[:, hf * D:(hf + 1) * D, 128:256].rearrange(
                            "p c (d h) -> p c d h", d=HD),
                        in1=bass.AP(tensor=eh.tensor, offset=eh.offset,
                                    ap=[[eh.ap[0][0], 128], [eh.ap[1][0], D],
                                        [0, HD], [1, H]]),
                        op=mybir.AluOpType.mult)

                ps_agg = psp.tile([128, C * H], f32, tag="sa")
                for cc in range(C):
                    nc.tensor.matmul(out=ps_agg[:, 0:136], lhsT=ohE_v[:, cc * 128:(cc + 1) * 128],
                                     rhs=wgt[:, cc, :], start=(cc == 0), stop=(cc == C - 1))

                # ---- finalize window: out = (num/den) @ Wo + bo (transposed)
                # (no +eps: every real node has in-degree >= 1; pad rows are
                # discarded by the host unshard)
                rec = winp.tile([128, H], f32, tag="rec")
                nc.vector.reciprocal(out=rec[:], in_=ps_agg[:, 128:136])
                h_sb = winp.tile([128, 128], b16, tag="hsb")
                rec_ap = rec[:]
                nc.vector.tensor_tensor(
                    out=h_sb[:].rearrange("p (d h) -> p d h", d=HD),
                    in0=ps_agg[:, 0:128].rearrange("p (d h) -> p d h", d=HD),
                    in1=bass.AP(tensor=rec_ap.tensor, offset=rec_ap.offset,
                                ap=[[rec_ap.ap[0][0], 128], [0, HD], [1, H]]),
                    op=mybir.AluOpType.mult)
                ps_T = psp.tile([128, 128], b16, tag="fin")
                nc.tensor.transpose(out=ps_T[:], in_=h_sb[:], identity=ident[:])
                hT_sb = winp.tile([128, 128], b16, tag="hTsb")
                nc.vector.tensor_copy(out=hT_sb[:], in_=ps_T[:])
                ps_out = psp.tile([128, 128], f32, tag="fin")
                nc.tensor.matmul(out=ps_out[:], lhsT=Wo_sb[:], rhs=hT_sb[:],
                                 start=True, stop=True)
                out_sb = winp.tile([128, 128], f32, tag="osb")
                nc.scalar.activation(out=out_sb[:], in_=ps_out[:],
                                     func=mybir.ActivationFunctionType.Identity,
                                     bias=bo_sb[:])
                nc.sync.dma_start(out=d_out[:, w * 128:(w + 1) * 128], in_=out_sb[:])
    nc.compile()
    return nc


# ----------------------------------------------------------------------------
# inline SPMD runner (self-contained; mirrors concourse.bass2jax.run_bass_via_pjrt)
# ----------------------------------------------------------------------------

def _run_spmd(nc, in_maps):
    import jax
    import numpy as _np
    from jax.sharding import Mesh, PartitionSpec
    from jax.experimental.shard_map import shard_map
    import concourse.mybir as mybir
    from concourse.bass2jax import install_neuronx_cc_hook, _bass_exec_p, partition_id_tensor

    install_neuronx_cc_hook()
    partition_name = nc.partition_id_tensor.name if nc.partition_id_tensor else None
    in_names, out_names, out_avals, zero_outs = [], [], [], []
    for alloc in nc.m.functions[0].allocations:
        if not isinstance(alloc, mybir.MemoryLocationSet):
            continue
        name = alloc.memorylocations[0].name
        if alloc.kind == "ExternalInput":
            if name != partition_name:
                in_names.append(name)
        elif alloc.kind == "ExternalOutput":
            out_names.append(name)
            shape = tuple(alloc.tensor_shape)
            dtype = mybir.dt.np(alloc.dtype)
            out_avals.append(jax.core.ShapedArray(shape, dtype))
            zero_outs.append(_np.zeros(shape, dtype))
    n_params = len(in_names)
    all_in_names = list(in_names) + list(out_names)
    if partition_name is not None:
        all_in_names.append(partition_name)

    def _body(*args):
        operands = list(args)
        if partition_name is not None:
            operands.append(partition_id_tensor())
        outs = _bass_exec_p.bind(
            *operands,
            out_avals=tuple(out_avals),
            in_names=tuple(all_in_names),
            out_names=tuple(out_names),
            lowering_input_output_aliases=(),
            sim_require_finite=False,
            sim_require_nnan=False,
            nc=nc,
        )
        return tuple(outs)

    donate = tuple(range(n_params, n_params + len(out_avals)))
    devices = jax.devices()[:N_CORES]
    mesh = Mesh(_np.asarray(devices), ("core",))
    in_specs = (PartitionSpec("core"),) * (n_params + len(out_avals))
    out_specs = (PartitionSpec("core"),) * len(out_names)
    fn = jax.jit(shard_map(_body, mesh=mesh, in_specs=in_specs,
                           out_specs=out_specs, check_rep=False),
                 donate_argnums=donate, keep_unused=True)
    ins = []
    for nm in in_names:
        cat = _np.concatenate([_np.asarray(m[nm]) for m in in_maps], axis=0)
        ins.append(jax.device_put(cat, jax.sharding.NamedSharding(mesh, PartitionSpec("core"))))
    zouts = []
    for z in zero_outs:
        cat = _np.concatenate([z] * N_CORES, axis=0)
        zouts.append(jax.device_put(cat, jax.sharding.NamedSharding(mesh, PartitionSpec("core"))))
    outs = fn(*ins, *zouts)
    outs = [_np.asarray(o) for o in outs]
    per_core = []
    for c in range(N_CORES):
        d = {}
        for i, nm in enumerate(out_names):
            full = outs[i]
            rows = full.shape[0] // N_CORES
            d[nm] = full[c * rows:(c + 1) * rows]
        per_core.append(d)
    return per_core


_CACHE = {}


def kernel(node_features, edge_index, edge_features,
           Wl, bl, Wr, br, We, be, attn_vector, Wv, bv, Wo, bo):
    host, per_core = _host_prepare(node_features, edge_index, edge_features,
                                   Wl, bl, Wr, br, We, be, attn_vector,
                                   Wv, bv, Wo, bo)
    D = host["D"]
    if D not in _CACHE:
        _CACHE[D] = _build_nc(D)
    nc = _CACHE[D]

    res = _run_spmd(nc, per_core)
    outT = np.concatenate([res[c]["outT"] for c in range(N_CORES)], axis=1)  # [128, NPAD]
    out_relab = outT.T  # [NPAD, 128]
    out = out_relab[host["new_id"][:N_NODES]]
    return np.ascontiguousarray(out, dtype=np.float32)
